# Initial kernel scaffold
#
"""Trainium2 Bass kernel for 12-head MHA (S=4096, D=768) on 8 NeuronCores.

Sharding: data-parallel over the query/sequence dim, with the K^T and V
projections tensor-sharded: each core projects K^T/V only for its own 512
sequence rows and one packed AllGather assembles the full K^T [768, 4096]
and ones-augmented V [4096, 12*65] on every core (~75us intra-chip, off the
PE/ACT critical path). Each core then runs attention for its 512 queries
over all 4096 keys and the output FC for its rows; outputs are disjoint row
blocks the host concatenates.

Layout tricks:
  - All matmul operands are pre-transposed on the host ([contraction, out])
    and cast to bf16, so nothing needs an on-chip transpose.
  - Scores are computed transposed (scoresT[kpos, q]) so that softmax(exp)
    output feeds attn@V directly as the moving operand (contraction = kpos
    on partitions). No max-subtraction (scores are ~N(0,1); exp is safe).
  - Softmax denominators come for free from a ones-column appended to V
    (row 64 of the attn@V accumulator); normalization happens on the tiny
    [65, 512] attention outputs, never on the 25M-element score tensor.
  - Head pairs share the 128-partition feature chunks; the two K=64 score
    matmuls of a pair run concurrently in disjoint PE row-groups.
  - exp runs on the scalar engine straight out of PSUM ([128,1024] per op,
    scale=1/8 folded into the activation's free affine), writing bf16.
    The scalar engine is the roofline: 25.2M exps/core ~ 191us.
"""

import numpy as np
import ml_dtypes

import concourse.bass as bass
import concourse.tile as tile
from concourse import mybir
import bass_rust

HID = 768
S = 4096
NCORES = 8
SC = S // NCORES          # 512 queries per core
HEADS = 12
HD = 64
PAIRS = HEADS // 2        # 6
NCH = HID // 128          # 6 feature chunks
NKT = S // 128            # 32 kpos blocks
NQB = SC // 128           # 4 query blocks
SCALE = 1.0 / 8.0         # 1/sqrt(64)

BF = mybir.dt.bfloat16
F32 = mybir.dt.float32
EXP = mybir.ActivationFunctionType.Exp

VW = HD + 1               # 65: V columns per head incl. ones column


def split_excess_waits(nc, max_waits=1):
    """This walrus build rejects >1 sem wait per instruction; move extras
    onto preceding NOPs on the same engine (same semantics: engine blocks
    until all waits pass before executing the original instruction)."""
    ctr = 0
    for fn in nc.m.functions:
        for bb in fn.blocks:
            new_list = []
            for ins in bb.instructions:
                si = ins.sync_info
                if si is not None and si.on_wait and len(si.on_wait) > max_waits:
                    waits = list(si.on_wait)
                    while len(waits) > max_waits:
                        chunk, waits = waits[:max_waits], waits[max_waits:]
                        nop = bass_rust.InstNoOp(
                            name=f"I-waitsplit-{ctr}", ins=[], outs=[])
                        ctr += 1
                        nop.engine = ins.engine
                        nop.sync_info = mybir.SyncInfo(on_wait=chunk, on_update=[])
                        new_list.append(nop)
                    ins.sync_info = mybir.SyncInfo(
                        on_wait=waits, on_update=list(si.on_update))
                new_list.append(ins)
            bb.instructions[:] = new_list
    return ctr


def build_nc(split_waits=True, repeats=1):
    nc = bass.Bass()
    xqT = nc.declare_dram_parameter("xqT", [HID, SC], BF, isOutput=False)
    wqT = nc.declare_dram_parameter("wqT", [HID, HID], BF, isOutput=False)
    wkT = nc.declare_dram_parameter("wkT", [HID, HID], BF, isOutput=False)
    wvT = nc.declare_dram_parameter("wvT", [HID, HID], BF, isOutput=False)
    wfcT = nc.declare_dram_parameter("wfcT", [HID, HID], BF, isOutput=False)
    out = nc.declare_dram_parameter("out", [SC, HID], F32, isOutput=True)

    with tile.TileContext(nc) as tc:
        with (
            tc.tile_pool(name="w", bufs=12) as p_w,        # wq/wk/wv share
            tc.tile_pool(name="xqw", bufs=NCH) as p_xqw,   # xq then wfc
            tc.tile_pool(name="KT", bufs=NCH) as p_KT,
            tc.tile_pool(name="V", bufs=NKT) as p_V,
            tc.tile_pool(name="QT", bufs=NCH) as p_QT,
            tc.tile_pool(name="oNT", bufs=NCH) as p_oNT,
            tc.tile_pool(name="expT", bufs=4) as p_exp,
            tc.tile_pool(name="norm", bufs=2) as p_norm,
            tc.tile_pool(name="ones", bufs=1) as p_ones,
            tc.tile_pool(name="osb", bufs=2) as p_osb,
            tc.tile_pool(name="dram", bufs=1, space="DRAM") as p_dram,
            tc.tile_pool(name="pp_mm", bufs=2, space="PSUM") as pp_mm,
            tc.tile_pool(name="pp_sc", bufs=2, space="PSUM") as pp_sc,
            tc.tile_pool(name="pp_acc", bufs=2, space="PSUM") as pp_acc,
        ):
            for _rep in range(repeats):
                # ---- input loads ----
                xq_sb = []
                for i in range(NCH):
                    t = p_xqw.tile([128, SC], BF, tag="xqw", name=f"xq{i}")
                    nc.sync.dma_start(out=t, in_=xqT[128 * i:128 * (i + 1), :])
                    xq_sb.append(t)

                def load_w(param, nm):
                    ts = []
                    for i in range(NCH):
                        t = p_w.tile([128, HID], BF, tag="w", name=f"{nm}{i}")
                        nc.sync.dma_start(
                            out=t, in_=param[128 * i:128 * (i + 1), :])
                        ts.append(t)
                    return ts

                ones_sb = p_ones.tile([1, HD], BF, tag="ones", name="ones")
                nc.vector.memset(ones_sb, 1.0)

                # ---- K^T and V local slices + one AllGather over 8 cores --
                # each core projects K^T/V only for its own 512 seq rows
                # (from xq); the AllGather assembles the full K^T [768,4096]
                # and V [4096, 12*65]. Emitted first so the gather runs under
                # the Q projection and the attention pipeline start.
                wk_sb = load_w(wkT, "wk")
                wv_sb = load_w(wvT, "wv")
                # one flat local buffer: [K^T local chunks | V local chunks]
                KLEN = NCH * 128 * SC
                VROW = HEADS * VW
                LLEN = KLEN + NQB * 128 * VROW
                kvl_dram = p_dram.tile([LLEN], BF, name=f"kvl{_rep}")
                for j in range(NCH):
                    ps = pp_mm.tile([128, 512], F32, tag="mm", name=f"pskl{j}")
                    for i in range(NCH):
                        nc.tensor.matmul(
                            ps,
                            lhsT=wk_sb[i][:, 128 * j:128 * (j + 1)],
                            rhs=xq_sb[i],
                            start=(i == 0), stop=(i == NCH - 1))
                    kl = p_QT.tile([128, SC], BF, tag="ktloc", bufs=NCH,
                                   name=f"ktloc{j}")
                    nc.vector.tensor_copy(kl, ps)
                    nc.sync.dma_start(
                        out=kvl_dram[j * 128 * SC:(j + 1) * 128 * SC]
                        .rearrange("(p n) -> p n", p=128),
                        in_=kl)
                for mloc in range(NQB):
                    vt = p_V.tile([128, VROW], BF, tag="vloc",
                                  bufs=2, name=f"vloc{mloc}")
                    v3 = vt.rearrange("p (h w) -> p h w", w=VW)
                    nc.vector.memset(v3[:, :, HD:], 1.0)
                    for half in range(2):
                        ps = pp_mm.tile([128, 384], F32, tag="mm",
                                        name=f"psvl{mloc}_{half}")
                        for i in range(NCH):
                            nc.tensor.matmul(
                                ps,
                                lhsT=xq_sb[i][:, 128 * mloc:128 * (mloc + 1)],
                                rhs=wv_sb[i][:, 384 * half:384 * (half + 1)],
                                start=(i == 0), stop=(i == NCH - 1))
                        nc.vector.tensor_copy(
                            v3[:, 6 * half:6 * (half + 1), 0:HD],
                            ps.rearrange("p (h w) -> p h w", w=HD))
                    base = KLEN + mloc * 128 * VROW
                    nc.sync.dma_start(
                        out=kvl_dram[base:base + 128 * VROW]
                        .rearrange("(p n) -> p n", p=128),
                        in_=vt)
                kvg_dram = p_dram.tile([NCORES * LLEN], BF,
                                       name=f"kvg{_rep}", addr_space="Shared")
                nc.gpsimd.collective_compute(
                    "AllGather",
                    mybir.AluOpType.bypass,
                    replica_groups=[list(range(NCORES))],
                    ins=[kvl_dram],
                    outs=[kvg_dram],
                )

                # ---- Q^T projection (overlaps the gather) ----
                wq_sb = load_w(wqT, "wq")
                QT_sb = []
                for j in range(NCH):
                    qt = p_QT.tile([128, SC], BF, tag="QT", name=f"QT{j}")
                    ps = pp_mm.tile([128, 512], F32, tag="mm", name=f"psq{j}")
                    for i in range(NCH):
                        nc.tensor.matmul(
                            ps,
                            lhsT=wq_sb[i][:, 128 * j:128 * (j + 1)],
                            rhs=xq_sb[i],
                            start=(i == 0), stop=(i == NCH - 1))
                    nc.vector.tensor_copy(qt, ps)
                    QT_sb.append(qt)

                def emit_K(j):
                    kt = p_KT.tile([128, S], BF, tag="KT", name=f"KT{j}")
                    # fetch gathered chunk j: kt[p, 512c+n] = core c's block
                    src = bass.AP(
                        tensor=kvg_dram.tensor,
                        offset=kvg_dram.offset + j * 128 * SC,
                        ap=[[SC, 128], [LLEN, NCORES], [1, SC]])
                    nc.sync.dma_start(
                        out=kt.rearrange("p (c n) -> p c n", n=SC), in_=src)
                    return kt

                # fetch gathered V (global s-chunk m = 4c + mloc)
                V_sb = []
                for m in range(NKT):
                    c, mloc = divmod(m, NQB)
                    vt = p_V.tile([128, VROW], BF, tag="V", name=f"V{m}")
                    src = bass.AP(
                        tensor=kvg_dram.tensor,
                        offset=(kvg_dram.offset + c * LLEN + KLEN
                                + mloc * 128 * VROW),
                        ap=[[VROW, 128], [1, VROW]])
                    nc.sync.dma_start(out=vt, in_=src)
                    V_sb.append(vt)

                oNT_sb = []

                def emit_pair(p, KT_p):
                    acc = [pp_acc.tile([128, SC], F32, tag="acc",
                                       name=f"acc{p}_{h}") for h in range(2)]
                    for t in range(NKT):
                        sc = pp_sc.tile([128, 2 * SC], F32, tag="sc",
                                        name=f"sc{p}_{t}")
                        for half in range(2):
                            nc.tensor.matmul(
                                sc[:, SC * half:SC * (half + 1)],
                                lhsT=KT_p[64 * half:64 * (half + 1),
                                          128 * t:128 * (t + 1)],
                                rhs=QT_sb[p][64 * half:64 * (half + 1), :],
                                start=True, stop=True)
                        et = p_exp.tile([128, 2 * SC], BF, tag="expT",
                                        name=f"et{p}_{t}")
                        nc.scalar.activation(et, sc, EXP, scale=SCALE)
                        for half in range(2):
                            h = 2 * p + half
                            nc.tensor.matmul(
                                acc[half][0:VW, :],
                                lhsT=V_sb[t][:, VW * h:VW * (h + 1)],
                                rhs=et[:, SC * half:SC * (half + 1)],
                                start=(t == 0), stop=(t == NKT - 1))
                    # drain accumulators to SBUF promptly so the PSUM banks
                    # free up for the next pair's attnV
                    accs = []
                    for half in range(2):
                        a = p_norm.tile([VW, SC], F32, tag="accs",
                                        name=f"accs{p}_{half}")
                        nc.vector.tensor_copy(a, acc[half][0:VW, :])
                        accs.append(a)
                    # normalize: out[d, q] / rowsum[q]; rowsum is accs row 64
                    ot = p_oNT.tile([128, SC], BF, tag="oNT", name=f"oNT{p}")
                    for half in range(2):
                        rc = p_norm.tile([1, SC], BF, tag="recip", bufs=2,
                                         name=f"rc{p}_{half}")
                        with nc.allow_low_precision(
                                reason="softmax denom reciprocal in bf16"):
                            nc.vector.reciprocal(rc, accs[half][HD:VW, :])
                        # broadcast [1,SC] -> [64,SC] via K=1 matmul with ones
                        rbp = pp_mm.tile([128, SC], F32, tag="mm",
                                         name=f"rbp{p}_{half}")
                        nc.tensor.matmul(rbp[0:HD, :], lhsT=ones_sb, rhs=rc,
                                         start=True, stop=True)
                        rb = p_norm.tile([64, SC], BF, tag="rb", bufs=2,
                                         name=f"rb{p}_{half}")
                        nc.vector.tensor_copy(rb, rbp[0:HD, :])
                        if half == 0:
                            nc.vector.tensor_mul(
                                ot[0:64, :], accs[half][0:HD, :], rb)
                        else:
                            tmp = p_norm.tile([64, SC], BF, tag="tmpB", bufs=2,
                                              name=f"tmp{p}")
                            nc.vector.tensor_mul(tmp, accs[half][0:HD, :], rb)
                            # partition shift 0:64 -> 64:128 via DMA
                            nc.sync.dma_start(out=ot[64:128, :], in_=tmp)
                    oNT_sb.append(ot)

                # interleaved schedule: attention pair p starts as soon as
                # K^T chunk p and the right V half exist; later K chunks and
                # the second V half fill PE while ACT works through exp
                KT0 = emit_K(0)
                emit_pair(0, KT0)
                KT1 = emit_K(1)
                emit_pair(1, KT1)
                KT2 = emit_K(2)
                emit_pair(2, KT2)
                KT3 = emit_K(3)
                emit_pair(3, KT3)
                KT4 = emit_K(4)
                emit_pair(4, KT4)
                KT5 = emit_K(5)
                emit_pair(5, KT5)

                # ---- FC ----
                wfc_sb = []
                for i in range(NCH):
                    t = p_xqw.tile([128, HID], BF, tag="xqw", name=f"wfc{i}")
                    nc.sync.dma_start(
                        out=t, in_=wfcT[128 * i:128 * (i + 1), :])
                    wfc_sb.append(t)
                for qb in range(NQB):
                    osb = p_osb.tile([128, HID], F32, tag="osb",
                                     name=f"osb{qb}")
                    for fh in range(2):
                        ps = pp_mm.tile([128, 384], F32, tag="mm",
                                        name=f"psf{qb}_{fh}")
                        for j in range(NCH):
                            nc.tensor.matmul(
                                ps,
                                lhsT=oNT_sb[j][:, 128 * qb:128 * (qb + 1)],
                                rhs=wfc_sb[j][:, 384 * fh:384 * (fh + 1)],
                                start=(j == 0), stop=(j == NCH - 1))
                        nc.vector.tensor_copy(
                            osb[:, 384 * fh:384 * (fh + 1)], ps)
                    nc.sync.dma_start(
                        out=out[128 * qb:128 * (qb + 1), :], in_=osb)

    if split_waits:
        split_excess_waits(nc)
    return nc


_NC_CACHE = None


def _get_nc():
    global _NC_CACHE
    if _NC_CACHE is None:
        _NC_CACHE = build_nc()
    return _NC_CACHE


def make_in_maps(x, w_q, w_k, w_v, w_fc):
    bf16 = ml_dtypes.bfloat16
    xT = np.ascontiguousarray(np.asarray(x, np.float32)[0].T).astype(bf16)
    ws = {
        "wqT": np.ascontiguousarray(np.asarray(w_q, np.float32).T).astype(bf16),
        "wkT": np.ascontiguousarray(np.asarray(w_k, np.float32).T).astype(bf16),
        "wvT": np.ascontiguousarray(np.asarray(w_v, np.float32).T).astype(bf16),
        "wfcT": np.ascontiguousarray(np.asarray(w_fc, np.float32).T).astype(bf16),
    }
    in_maps = []
    for c in range(NCORES):
        m = {"xqT": np.ascontiguousarray(xT[:, SC * c:SC * (c + 1)])}
        m.update(ws)
        in_maps.append(m)
    return in_maps


def kernel(x, w_q, w_k, w_v, w_fc):
    from concourse.bass_utils import run_bass_kernel_spmd
    nc = _get_nc()
    in_maps = make_in_maps(x, w_q, w_k, w_v, w_fc)
    res = run_bass_kernel_spmd(nc, in_maps, core_ids=list(range(NCORES)))
    out = np.concatenate([res.results[c]["out"] for c in range(NCORES)], axis=0)
    return out.reshape(1, S, HID).astype(np.float32)



# revision 1
# speedup vs baseline: 1.4219x; 1.4219x over previous
"""Trainium2 Bass kernel for 12-head MHA (S=4096, D=768) on 8 NeuronCores.

Sharding: data-parallel over the query/sequence dim, with the K^T and V
projections tensor-sharded: each core projects K^T/V only for its own 512
sequence rows and one packed AllGather assembles the full K^T [768, 4096]
and ones-augmented V [4096, 12*65] on every core (~75us intra-chip, off the
PE/ACT critical path). Each core then runs attention for its 512 queries
over all 4096 keys and the output FC for its rows; outputs are disjoint row
blocks the host concatenates.

Layout tricks:
  - All matmul operands are pre-transposed on the host ([contraction, out])
    and cast to bf16, so nothing needs an on-chip transpose.
  - Scores are computed transposed (scoresT[kpos, q]) so that softmax(exp)
    output feeds attn@V directly as the moving operand (contraction = kpos
    on partitions). No max-subtraction (scores are ~N(0,1); exp is safe).
  - Softmax denominators come for free from a ones-column appended to V
    (row 64 of the attn@V accumulator); normalization happens on the tiny
    [65, 512] attention outputs, never on the 25M-element score tensor.
  - Head pairs share the 128-partition feature chunks; the two K=64 score
    matmuls of a pair run concurrently in disjoint PE row-groups.
  - exp runs on the scalar engine straight out of PSUM ([128,1024] per op,
    scale=1/8 folded into the activation's free affine), writing bf16.
    The scalar engine is the roofline: 25.2M exps/core ~ 191us.
"""

import numpy as np
import ml_dtypes

import concourse.bass as bass
import concourse.tile as tile
from concourse import mybir
import bass_rust

HID = 768
S = 4096
NCORES = 8
SC = S // NCORES          # 512 queries per core
HEADS = 12
HD = 64
PAIRS = HEADS // 2        # 6
NCH = HID // 128          # 6 feature chunks
NKT = S // 128            # 32 kpos blocks
NQB = SC // 128           # 4 query blocks
SCALE = 1.0 / 8.0         # 1/sqrt(64)

BF = mybir.dt.bfloat16
F32 = mybir.dt.float32
EXP = mybir.ActivationFunctionType.Exp

VW = HD + 1               # 65: V columns per head incl. ones column


def split_excess_waits(nc, max_waits=1):
    """This walrus build rejects >1 sem wait per instruction; move extras
    onto preceding NOPs on the same engine (same semantics: engine blocks
    until all waits pass before executing the original instruction)."""
    ctr = 0
    for fn in nc.m.functions:
        for bb in fn.blocks:
            new_list = []
            for ins in bb.instructions:
                si = ins.sync_info
                if si is not None and si.on_wait and len(si.on_wait) > max_waits:
                    waits = list(si.on_wait)
                    while len(waits) > max_waits:
                        chunk, waits = waits[:max_waits], waits[max_waits:]
                        nop = bass_rust.InstNoOp(
                            name=f"I-waitsplit-{ctr}", ins=[], outs=[])
                        ctr += 1
                        nop.engine = ins.engine
                        nop.sync_info = mybir.SyncInfo(on_wait=chunk, on_update=[])
                        new_list.append(nop)
                    ins.sync_info = mybir.SyncInfo(
                        on_wait=waits, on_update=list(si.on_update))
                new_list.append(ins)
            bb.instructions[:] = new_list
    return ctr


def build_nc(split_waits=True, repeats=1):
    nc = bass.Bass()
    xqT = nc.declare_dram_parameter("xqT", [HID, SC], BF, isOutput=False)
    wqT = nc.declare_dram_parameter("wqT", [HID, HID], BF, isOutput=False)
    wkT = nc.declare_dram_parameter("wkT", [HID, HID], BF, isOutput=False)
    wvT = nc.declare_dram_parameter("wvT", [HID, HID], BF, isOutput=False)
    wfcT = nc.declare_dram_parameter("wfcT", [HID, HID], BF, isOutput=False)
    out = nc.declare_dram_parameter("out", [SC, HID], F32, isOutput=True)

    with tile.TileContext(nc) as tc:
        with (
            tc.tile_pool(name="w", bufs=12) as p_w,        # wq/wk/wv share
            tc.tile_pool(name="xqw", bufs=NCH) as p_xqw,   # xq then wfc
            tc.tile_pool(name="KT", bufs=NCH) as p_KT,
            tc.tile_pool(name="V", bufs=NKT) as p_V,
            tc.tile_pool(name="QT", bufs=NCH) as p_QT,
            tc.tile_pool(name="oNT", bufs=NCH) as p_oNT,
            tc.tile_pool(name="expT", bufs=4) as p_exp,
            tc.tile_pool(name="norm", bufs=2) as p_norm,
            tc.tile_pool(name="ones", bufs=1) as p_ones,
            tc.tile_pool(name="osb", bufs=2) as p_osb,
            tc.tile_pool(name="dram", bufs=1, space="DRAM") as p_dram,
            tc.tile_pool(name="pp_mm", bufs=2, space="PSUM") as pp_mm,
            tc.tile_pool(name="pp_sc", bufs=2, space="PSUM") as pp_sc,
            tc.tile_pool(name="pp_acc", bufs=2, space="PSUM") as pp_acc,
        ):
            for _rep in range(repeats):
                # ---- input loads ----
                xq_sb = []
                for i in range(NCH):
                    t = p_xqw.tile([128, SC], BF, tag="xqw", name=f"xq{i}")
                    nc.sync.dma_start(out=t, in_=xqT[128 * i:128 * (i + 1), :])
                    xq_sb.append(t)

                def load_w(param, nm):
                    ts = []
                    for i in range(NCH):
                        t = p_w.tile([128, HID], BF, tag="w", name=f"{nm}{i}")
                        nc.sync.dma_start(
                            out=t, in_=param[128 * i:128 * (i + 1), :])
                        ts.append(t)
                    return ts

                ones_sb = p_ones.tile([1, HD], BF, tag="ones", name="ones")
                nc.vector.memset(ones_sb, 1.0)

                # ---- K^T and V local slices + one AllGather over 8 cores --
                # each core projects K^T/V only for its own 512 seq rows
                # (from xq); the AllGather assembles the full K^T [768,4096]
                # and V [4096, 12*65]. Emitted first so the gather runs under
                # the Q projection and the attention pipeline start.
                wk_sb = load_w(wkT, "wk")
                wv_sb = load_w(wvT, "wv")
                # one flat local buffer: [K^T local chunks | V local chunks]
                KLEN = NCH * 128 * SC
                VROW = HEADS * VW
                LLEN = KLEN + NQB * 128 * VROW
                kvl_dram = p_dram.tile([LLEN], BF, name=f"kvl{_rep}")
                for j in range(NCH):
                    ps = pp_mm.tile([128, 512], F32, tag="mm", name=f"pskl{j}")
                    for i in range(NCH):
                        nc.tensor.matmul(
                            ps,
                            lhsT=wk_sb[i][:, 128 * j:128 * (j + 1)],
                            rhs=xq_sb[i],
                            start=(i == 0), stop=(i == NCH - 1))
                    kl = p_QT.tile([128, SC], BF, tag="ktloc", bufs=NCH,
                                   name=f"ktloc{j}")
                    nc.vector.tensor_copy(kl, ps)
                    nc.sync.dma_start(
                        out=kvl_dram[j * 128 * SC:(j + 1) * 128 * SC]
                        .rearrange("(p n) -> p n", p=128),
                        in_=kl)
                for mloc in range(NQB):
                    vt = p_V.tile([128, VROW], BF, tag="vloc",
                                  bufs=2, name=f"vloc{mloc}")
                    v3 = vt.rearrange("p (h w) -> p h w", w=VW)
                    nc.vector.memset(v3[:, :, HD:], 1.0)
                    for half in range(2):
                        ps = pp_mm.tile([128, 384], F32, tag="mm",
                                        name=f"psvl{mloc}_{half}")
                        for i in range(NCH):
                            nc.tensor.matmul(
                                ps,
                                lhsT=xq_sb[i][:, 128 * mloc:128 * (mloc + 1)],
                                rhs=wv_sb[i][:, 384 * half:384 * (half + 1)],
                                start=(i == 0), stop=(i == NCH - 1))
                        nc.vector.tensor_copy(
                            v3[:, 6 * half:6 * (half + 1), 0:HD],
                            ps.rearrange("p (h w) -> p h w", w=HD))
                    base = KLEN + mloc * 128 * VROW
                    nc.sync.dma_start(
                        out=kvl_dram[base:base + 128 * VROW]
                        .rearrange("(p n) -> p n", p=128),
                        in_=vt)
                kvg_dram = p_dram.tile([NCORES * LLEN], BF,
                                       name=f"kvg{_rep}", addr_space="Shared")
                nc.gpsimd.collective_compute(
                    "AllGather",
                    mybir.AluOpType.bypass,
                    replica_groups=[list(range(NCORES))],
                    ins=[kvl_dram],
                    outs=[kvg_dram],
                )

                # ---- Q^T projection (overlaps the gather) ----
                wq_sb = load_w(wqT, "wq")
                QT_sb = []
                for j in range(NCH):
                    qt = p_QT.tile([128, SC], BF, tag="QT", name=f"QT{j}")
                    ps = pp_mm.tile([128, 512], F32, tag="mm", name=f"psq{j}")
                    for i in range(NCH):
                        nc.tensor.matmul(
                            ps,
                            lhsT=wq_sb[i][:, 128 * j:128 * (j + 1)],
                            rhs=xq_sb[i],
                            start=(i == 0), stop=(i == NCH - 1))
                    nc.vector.tensor_copy(qt, ps)
                    QT_sb.append(qt)

                def emit_K(j):
                    kt = p_KT.tile([128, S], BF, tag="KT", name=f"KT{j}")
                    # fetch gathered chunk j: kt[p, 512c+n] = core c's block
                    src = bass.AP(
                        tensor=kvg_dram.tensor,
                        offset=kvg_dram.offset + j * 128 * SC,
                        ap=[[SC, 128], [LLEN, NCORES], [1, SC]])
                    nc.sync.dma_start(
                        out=kt.rearrange("p (c n) -> p c n", n=SC), in_=src)
                    return kt

                # fetch gathered V (global s-chunk m = 4c + mloc)
                V_sb = []
                for m in range(NKT):
                    c, mloc = divmod(m, NQB)
                    vt = p_V.tile([128, VROW], BF, tag="V", name=f"V{m}")
                    src = bass.AP(
                        tensor=kvg_dram.tensor,
                        offset=(kvg_dram.offset + c * LLEN + KLEN
                                + mloc * 128 * VROW),
                        ap=[[VROW, 128], [1, VROW]])
                    nc.sync.dma_start(out=vt, in_=src)
                    V_sb.append(vt)

                oNT_sb = []

                def emit_pair(p, KT_p):
                    acc = [pp_acc.tile([128, SC], F32, tag="acc",
                                       name=f"acc{p}_{h}") for h in range(2)]
                    for t in range(NKT):
                        sc = pp_sc.tile([128, 2 * SC], F32, tag="sc",
                                        name=f"sc{p}_{t}")
                        for half in range(2):
                            nc.tensor.matmul(
                                sc[:, SC * half:SC * (half + 1)],
                                lhsT=KT_p[64 * half:64 * (half + 1),
                                          128 * t:128 * (t + 1)],
                                rhs=QT_sb[p][64 * half:64 * (half + 1), :],
                                start=True, stop=True)
                        et = p_exp.tile([128, 2 * SC], BF, tag="expT",
                                        name=f"et{p}_{t}")
                        nc.scalar.activation(et, sc, EXP, scale=SCALE)
                        for half in range(2):
                            h = 2 * p + half
                            nc.tensor.matmul(
                                acc[half][0:VW, :],
                                lhsT=V_sb[t][:, VW * h:VW * (h + 1)],
                                rhs=et[:, SC * half:SC * (half + 1)],
                                start=(t == 0), stop=(t == NKT - 1))
                    # drain accumulators to SBUF promptly so the PSUM banks
                    # free up for the next pair's attnV
                    accs = []
                    for half in range(2):
                        a = p_norm.tile([VW, SC], F32, tag="accs",
                                        name=f"accs{p}_{half}")
                        nc.vector.tensor_copy(a, acc[half][0:VW, :])
                        accs.append(a)
                    # normalize: out[d, q] / rowsum[q]; rowsum is accs row 64
                    ot = p_oNT.tile([128, SC], BF, tag="oNT", name=f"oNT{p}")
                    for half in range(2):
                        rc = p_norm.tile([1, SC], BF, tag="recip", bufs=2,
                                         name=f"rc{p}_{half}")
                        with nc.allow_low_precision(
                                reason="softmax denom reciprocal in bf16"):
                            nc.vector.reciprocal(rc, accs[half][HD:VW, :])
                        # broadcast [1,SC] -> [64,SC] via K=1 matmul with ones
                        rbp = pp_mm.tile([128, SC], F32, tag="mm",
                                         name=f"rbp{p}_{half}")
                        nc.tensor.matmul(rbp[0:HD, :], lhsT=ones_sb, rhs=rc,
                                         start=True, stop=True)
                        rb = p_norm.tile([64, SC], BF, tag="rb", bufs=2,
                                         name=f"rb{p}_{half}")
                        nc.vector.tensor_copy(rb, rbp[0:HD, :])
                        if half == 0:
                            nc.vector.tensor_mul(
                                ot[0:64, :], accs[half][0:HD, :], rb)
                        else:
                            tmp = p_norm.tile([64, SC], BF, tag="tmpB", bufs=2,
                                              name=f"tmp{p}")
                            nc.vector.tensor_mul(tmp, accs[half][0:HD, :], rb)
                            # partition shift 0:64 -> 64:128 via DMA
                            nc.sync.dma_start(out=ot[64:128, :], in_=tmp)
                    oNT_sb.append(ot)

                # interleaved schedule: attention pair p starts as soon as
                # K^T chunk p and the right V half exist; later K chunks and
                # the second V half fill PE while ACT works through exp
                KT0 = emit_K(0)
                emit_pair(0, KT0)
                KT1 = emit_K(1)
                emit_pair(1, KT1)
                KT2 = emit_K(2)
                emit_pair(2, KT2)
                KT3 = emit_K(3)
                emit_pair(3, KT3)
                KT4 = emit_K(4)
                emit_pair(4, KT4)
                KT5 = emit_K(5)
                emit_pair(5, KT5)

                # ---- FC ----
                wfc_sb = []
                for i in range(NCH):
                    t = p_xqw.tile([128, HID], BF, tag="xqw", name=f"wfc{i}")
                    nc.sync.dma_start(
                        out=t, in_=wfcT[128 * i:128 * (i + 1), :])
                    wfc_sb.append(t)
                for qb in range(NQB):
                    osb = p_osb.tile([128, HID], F32, tag="osb",
                                     name=f"osb{qb}")
                    for fh in range(2):
                        ps = pp_mm.tile([128, 384], F32, tag="mm",
                                        name=f"psf{qb}_{fh}")
                        for j in range(NCH):
                            nc.tensor.matmul(
                                ps,
                                lhsT=oNT_sb[j][:, 128 * qb:128 * (qb + 1)],
                                rhs=wfc_sb[j][:, 384 * fh:384 * (fh + 1)],
                                start=(j == 0), stop=(j == NCH - 1))
                        nc.vector.tensor_copy(
                            osb[:, 384 * fh:384 * (fh + 1)], ps)
                    nc.sync.dma_start(
                        out=out[128 * qb:128 * (qb + 1), :], in_=osb)

    if split_waits:
        split_excess_waits(nc)
    return nc


_NC_CACHE = None


def _get_nc():
    global _NC_CACHE
    if _NC_CACHE is None:
        _NC_CACHE = build_nc()
    return _NC_CACHE


def make_in_maps(x, w_q, w_k, w_v, w_fc):
    bf16 = ml_dtypes.bfloat16
    xT = np.ascontiguousarray(np.asarray(x, np.float32)[0].T).astype(bf16)
    ws = {
        "wqT": np.ascontiguousarray(np.asarray(w_q, np.float32).T).astype(bf16),
        "wkT": np.ascontiguousarray(np.asarray(w_k, np.float32).T).astype(bf16),
        "wvT": np.ascontiguousarray(np.asarray(w_v, np.float32).T).astype(bf16),
        "wfcT": np.ascontiguousarray(np.asarray(w_fc, np.float32).T).astype(bf16),
    }
    in_maps = []
    for c in range(NCORES):
        m = {"xqT": np.ascontiguousarray(xT[:, SC * c:SC * (c + 1)])}
        m.update(ws)
        in_maps.append(m)
    return in_maps


def kernel(x, w_q, w_k, w_v, w_fc):
    from concourse.bass_utils import run_bass_kernel_spmd
    nc = _get_nc()
    in_maps = make_in_maps(x, w_q, w_k, w_v, w_fc)
    res = run_bass_kernel_spmd(nc, in_maps, core_ids=list(range(NCORES)))
    out = np.concatenate([res.results[c]["out"] for c in range(NCORES)], axis=0)
    return out.reshape(1, S, HID).astype(np.float32)

